# revision 2
# baseline (speedup 1.0000x reference)
"""Trainium2 Bass kernel for nn_Bert_AvgPooling (segment_reduce + mean + FC).

reference semantics:
    tokens = sequence_output.reshape(B*S, H)              # [32768, 768]
    sums   = segment_sum(tokens, seg_ids, 1537)           # sentinel id 1536
    mean   = sums[:1536] / clause_counts[:, None]
    logits = mean @ fc_w.T + fc_b                         # [1536, 16]

Strategy (8 cores, sharded at clause boundaries => no collective):
  - seg ids are non-decreasing over masked positions, so clauses occupy
    contiguous token ranges.  Core c owns clauses [192c, 192c+192) and
    streams the contiguous token span covering them (sentinel tokens in
    between are harmless: their one-hot rows are all zero).
  - Two persistent PSUM accumulators per core: window A = clauses
    [c0, c0+128), window B = [c0+128, c0+256).  Per 128-token tile a
    one-hot (token x window-clause) is built on DVE with is_equal
    against an iota row; PE matmuls accumulate  psum[c,h] += oh.T @ tok.
    Tiles < NA feed window A, tiles >= NB0 feed window B (bounds are
    compile-time maxima over cores; out-of-window ids match nothing).
  - Windows evacuate through bf16 -> PE transpose -> FC matmul (768->16)
    -> scale by 1/count -> +bias -> plain DMA to this core's 192 output
    rows.  Host just concatenates the 8 shards.  Rows >= 192 of window B
    (neighbor cores' clauses, sentinel) are simply not written out.
"""

import sys

for _p in ("/opt/trn_rl_repo", "/opt/trn_rl_repo/concourse"):
    if _p not in sys.path:
        sys.path.insert(0, _p)

import numpy as np

import concourse.bacc as bacc
import concourse.mybir as mybir
import concourse.tile as tile
from concourse import bass_utils

F32 = mybir.dt.float32
BF16 = mybir.dt.bfloat16

B, S, H, NC = 64, 512, 768, 1536
CORES = 8
OUTR = 256  # output rows per core (window A 128 + window B 128); host slices
PAD_ID = 100000.0

LAST_EXEC_INFO = {}

_PROGRAM_CACHE = {}


def _build(NT2, NA, NB0, loop_iters=0, chunk=4, dma_mode="swdge_split", evac_mode="par"):
    """One program for all cores. NT2 = token tiles per core; window-A
    matmuls for tiles [0, NA); window-B matmuls for tiles [NB0, NT2).

    dma_mode: 'swdge_split' | 'swdge_unsplit' | 'hwdge_act' | 'hwdge_dve'
    | 'hwdge_hybrid'"""
    nc = bacc.Bacc(
        "TRN2",
        target_bir_lowering=False,
        debug=False,
        enable_asserts=False,
        num_devices=CORES,
    )
    tok_d = nc.dram_tensor("tok", [NT2 * 128, H], F32, kind="ExternalInput")
    rel_d = nc.dram_tensor("rel", [128, NT2], F32, kind="ExternalInput")
    fcw_d = nc.dram_tensor("fcw", [128, 6, 16], F32, kind="ExternalInput")
    fcb_d = nc.dram_tensor("fcb", [128, 16], F32, kind="ExternalInput")
    invc_d = nc.dram_tensor("invc", [128, 2], F32, kind="ExternalInput")
    out_d = nc.dram_tensor("out", [OUTR, 16], F32, kind="ExternalOutput")

    from contextlib import ExitStack
    import contextlib

    with tile.TileContext(nc) as tc, ExitStack() as ctx:
        cpool = ctx.enter_context(tc.tile_pool(name="const", bufs=1))
        # rel gates the one-hot builds (and thereby the PE): load it FIRST
        rel_s = cpool.tile([128, NT2], F32)
        nc.sync.dma_start(out=rel_s[:], in_=rel_d[:])
        # iota/identity are generated on-device; no DMA on the critical path
        iota_s = cpool.tile([128, 256], F32)
        nc.gpsimd.iota(
            iota_s[:], [[1, 256]], channel_multiplier=0,
            allow_small_or_imprecise_dtypes=True,
        )
        rowidx = cpool.tile([128, 1], F32)
        nc.gpsimd.iota(
            rowidx[:], [[1, 1]], channel_multiplier=1,
            allow_small_or_imprecise_dtypes=True,
        )
        ident = cpool.tile([128, 128], BF16)
        nc.vector.tensor_scalar(
            out=ident[:], in0=iota_s[:, :128], scalar1=rowidx[:, :1],
            scalar2=None, op0=mybir.AluOpType.is_equal,
        )
        fcw_f = cpool.tile([128, 6, 16], F32)
        nc.sync.dma_start(out=fcw_f[:], in_=fcw_d[:])
        fcw_s = cpool.tile([128, 6, 16], BF16)
        nc.scalar.copy(fcw_s[:], fcw_f[:])
        fcb_s = cpool.tile([128, 16], F32)
        nc.sync.dma_start(out=fcb_s[:], in_=fcb_d[:])
        invc_s = cpool.tile([128, 2], F32)
        nc.sync.dma_start(out=invc_s[:], in_=invc_d[:])

        CH = chunk
        hwdge = dma_mode.startswith("hwdge")
        if hwdge:
            tokfp = ctx.enter_context(tc.tile_pool(name="tokf", bufs=max(3, 16 // CH)))
        bfp = ctx.enter_context(
            tc.tile_pool(name="tokb", bufs=max(4, (24 if hwdge else 48) // CH))
        )
        ohp = ctx.enter_context(tc.tile_pool(name="oh", bufs=8))
        smallp = ctx.enter_context(tc.tile_pool(name="small", bufs=4))
        evacp = ctx.enter_context(tc.tile_pool(name="evac", bufs=2))
        psW = ctx.enter_context(tc.tile_pool(name="psW", bufs=1, space="PSUM"))
        psT = ctx.enter_context(tc.tile_pool(name="psT", bufs=1, space="PSUM"))
        psF = ctx.enter_context(tc.tile_pool(name="psF", bufs=1, space="PSUM"))

        def evac_serial(ps, wslot):
            sums_bf = evacp.tile([128, H], BF16, tag=f"sums{wslot}")
            pst = psT.tile([128, H], BF16, tag=f"psT{wslot}", space="PSUM")
            sumsT = evacp.tile([128, H], BF16, tag=f"sumsT{wslot}")
            psf = psF.tile([128, 32], F32, tag=f"psF{wslot}", space="PSUM")
            for k in range(6):
                sl = slice(k * 128, (k + 1) * 128)
                nc.scalar.copy(sums_bf[:, sl], ps[:, sl])
                nc.tensor.transpose(pst[:, sl], sums_bf[:, sl], ident[:])
                nc.vector.tensor_copy(sumsT[:, sl], pst[:, sl])
                nc.tensor.matmul(
                    psf[:, :16], sumsT[:, sl], fcw_s[:, k, :],
                    start=(k == 0), stop=(k == 5),
                )
            lg = smallp.tile([128, 16], F32, tag=f"lg{wslot}")
            nc.vector.tensor_scalar(
                out=lg[:], in0=psf[:, :16],
                scalar1=invc_s[:, wslot : wslot + 1], scalar2=None,
                op0=mybir.AluOpType.mult,
            )
            nc.vector.tensor_add(lg[:], lg[:], fcb_s[:])
            nc.sync.dma_start(
                out=out_d[wslot * 128 : (wslot + 1) * 128, :], in_=lg[:]
            )

        def evac_sums(ps, wslot):
            """PSUM window -> FC psum [128,16].  Window A evacuates
            mid-stream: keep its copies OFF the DVE queue (DVE feeds one-hots
            for later chunks; a DVE-queued copy waiting on the A-chain stop
            would stall them and starve the PE).  Window B evacuates at the
            end when DVE is free, so its copies alternate ACT/DVE."""
            use_dve = wslot == 1
            sums_bf = evacp.tile([128, H], BF16, tag=f"sums{wslot}")
            pst = psT.tile([128, H], BF16, tag=f"psT{wslot}", space="PSUM")
            sumsT = evacp.tile([128, H], BF16, tag=f"sumsT{wslot}")
            psf = psF.tile([128, 32], F32, tag=f"psF{wslot}", space="PSUM")
            for k in range(6):
                sl = slice(k * 128, (k + 1) * 128)
                if use_dve and k % 2 == 1:
                    nc.vector.tensor_copy(sums_bf[:, sl], ps[:, sl])
                else:
                    nc.scalar.copy(sums_bf[:, sl], ps[:, sl])
                nc.tensor.transpose(pst[:, sl], sums_bf[:, sl], ident[:])
                if use_dve and k % 2 == 0:
                    nc.vector.tensor_copy(sumsT[:, sl], pst[:, sl])
                else:
                    nc.scalar.copy(sumsT[:, sl], pst[:, sl])
                nc.tensor.matmul(
                    psf[:, :16], sumsT[:, sl], fcw_s[:, k, :],
                    start=(k == 0), stop=(k == 5),
                )
            return psf

        def finalize(psf, wslot):
            lg = smallp.tile([128, 16], F32, tag=f"lg{wslot}")
            nc.vector.tensor_scalar(
                out=lg[:], in0=psf[:, :16],
                scalar1=invc_s[:, wslot : wslot + 1], scalar2=None,
                op0=mybir.AluOpType.mult,
            )
            nc.vector.tensor_add(lg[:], lg[:], fcb_s[:])
            nc.sync.dma_start(
                out=out_d[wslot * 128 : (wslot + 1) * 128, :], in_=lg[:]
            )

        def evac_par(ps, wslot):
            finalize(evac_sums(ps, wslot), wslot)

        evac = evac_par if evac_mode == "par" else evac_serial

        loop_cm = tc.For_i(0, loop_iters, 1) if loop_iters else contextlib.nullcontext()
        with loop_cm:
            psA = psW.tile([128, H], F32, tag="psA", space="PSUM")
            psB = psW.tile([128, H], F32, tag="psB", space="PSUM")
            for t0 in range(0, NT2, CH):
                w = min(CH, NT2 - t0)
                tb = bfp.tile([128, CH, H], BF16, tag="tokb")
                src = tok_d[t0 * 128 : (t0 + w) * 128, :].rearrange(
                    "(c p) h -> p c h", p=128
                )
                ci = t0 // CH
                if dma_mode == "swdge_split":
                    # f32 -> bf16 cast happens inside the DMA engine (SWDGE)
                    nc.gpsimd.dma_start(
                        out=tb[:, :w, : H // 2], in_=src[:, :, : H // 2]
                    )
                    nc.gpsimd.dma_start(
                        out=tb[:, :w, H // 2 :], in_=src[:, :, H // 2 :]
                    )
                elif dma_mode == "swdge_unsplit":
                    nc.gpsimd.dma_start(out=tb[:, :w, :], in_=src)
                else:
                    tf = tokfp.tile([128, CH, H], F32, tag="tokf")
                    nc.sync.dma_start(out=tf[:, :w, : H // 2], in_=src[:, :, : H // 2])
                    nc.sync.dma_start(out=tf[:, :w, H // 2 :], in_=src[:, :, H // 2 :])
                    if dma_mode == "hwdge_act" or (
                        dma_mode == "hwdge_hybrid" and ci % 2 == 0
                    ):
                        nc.scalar.copy(tb[:, :w, :], tf[:, :w, :])
                    else:
                        nc.vector.tensor_copy(tb[:, :w, :], tf[:, :w, :])
                doA = t0 < NA
                doB = t0 + w > NB0
                if doA:
                    ohA = ohp.tile([128, CH, 128], BF16, tag="ohA")
                    nc.vector.tensor_tensor(
                        out=ohA[:, :w, :],
                        in0=rel_s[:, t0 : t0 + w, None].to_broadcast([128, w, 128]),
                        in1=iota_s[:, None, :128].to_broadcast([128, w, 128]),
                        op=mybir.AluOpType.is_equal,
                    )
                if doB:
                    ohB = ohp.tile([128, CH, 128], BF16, tag="ohB")
                    nc.vector.tensor_tensor(
                        out=ohB[:, :w, :],
                        in0=rel_s[:, t0 : t0 + w, None].to_broadcast([128, w, 128]),
                        in1=iota_s[:, None, 128:].to_broadcast([128, w, 128]),
                        op=mybir.AluOpType.is_equal,
                    )
                for i in range(w):
                    t = t0 + i
                    if t < NA:
                        nc.tensor.matmul(
                            psA[:, :512], ohA[:, i, :], tb[:, i, :512],
                            start=(t == 0), stop=(t == NA - 1),
                        )
                        nc.tensor.matmul(
                            psA[:, 512:], ohA[:, i, :], tb[:, i, 512:],
                            start=(t == 0), stop=(t == NA - 1),
                        )
                    if t >= NB0:
                        nc.tensor.matmul(
                            psB[:, :512], ohB[:, i, :], tb[:, i, :512],
                            start=(t == NB0), stop=(t == NT2 - 1),
                        )
                        nc.tensor.matmul(
                            psB[:, 512:], ohB[:, i, :], tb[:, i, 512:],
                            start=(t == NB0), stop=(t == NT2 - 1),
                        )
                if t0 < NA <= t0 + w:
                    # high priority so the scheduler interleaves window A's
                    # evacuation with the remaining stream instead of
                    # pushing it past the last chunk
                    if evac_mode == "par":
                        with tc.high_priority():
                            psfA = evac_sums(psA, 0)
                    else:
                        evac(psA, 0)
            if evac_mode == "par":
                psfB = evac_sums(psB, 1)
                finalize(psfA, 0)
                finalize(psfB, 1)
            else:
                evac(psB, 1)

    nc.compile()
    return nc


def _prepare(tok, seg, counts, fc_w, fc_b):
    """Host-side metadata: per-core token spans aligned to clause ranges,
    with split clauses chosen to balance span lengths across cores."""
    masked = seg < NC
    ids_m = seg[masked]
    sorted_ok = bool(np.all(np.diff(ids_m) >= 0)) and ids_m.size > 0
    if not sorted_ok:
        # arbitrary seg_ids: materialize tokens grouped by clause id
        order = np.argsort(ids_m, kind="stable")
        pos = np.flatnonzero(masked)[order]
        tok = np.ascontiguousarray(tok[pos])
        seg = ids_m[order]
        masked = np.ones(tok.shape[0], dtype=bool)
    ntok = tok.shape[0]
    mpos = np.flatnonzero(masked)
    idsall = seg[mpos]  # sorted clause id per masked token

    # balanced split clauses: core c covers clauses [splits[c], splits[c+1])
    nm = mpos.size
    splits = [0]
    for c in range(1, CORES):
        tgt = (c * nm) // CORES
        splits.append(int(idsall[min(tgt, nm - 1)]))
    splits.append(NC)
    # ensure strictly increasing (degenerate data)
    for c in range(1, CORES + 1):
        if splits[c] <= splits[c - 1]:
            splits[c] = min(NC, splits[c - 1] + 1)
    cnts = [splits[c + 1] - splits[c] for c in range(CORES)]
    if max(cnts) > OUTR:
        # fall back to uniform clause split
        splits = [c * (NC // CORES) for c in range(CORES)] + [NC]
        cnts = [splits[c + 1] - splits[c] for c in range(CORES)]

    starts, ends = [], []
    for c in range(CORES):
        lo_i = np.searchsorted(idsall, splits[c], side="left")
        hi_i = np.searchsorted(idsall, splits[c + 1], side="left")
        if lo_i == hi_i:
            starts.append(0)
            ends.append(1)
        else:
            starts.append(int(mpos[lo_i]))
            ends.append(int(mpos[hi_i - 1]) + 1)
    spans = [max(1, e - s) for s, e in zip(starts, ends)]
    NT2 = max((sp + 127) // 128 for sp in spans)

    counts_pad = np.ones(NC + 512, dtype=np.float32)
    counts_pad[:NC] = counts
    fcw = np.ascontiguousarray(fc_w.reshape(16, 6, 128).transpose(2, 1, 0))
    fcb = np.broadcast_to(fc_b[None, :], (128, 16)).copy()

    in_maps = []
    NA_max, NB0_min = 1, NT2 - 1
    for c in range(CORES):
        s = starts[c]
        c0 = splits[c]
        need = NT2 * 128
        if s + need <= ntok:
            tok_c = tok[s : s + need]
            rel_flat = seg[s : s + need].astype(np.float32) - c0
        else:
            tok_c = np.zeros((need, H), dtype=np.float32)
            avail = ntok - s
            tok_c[:avail] = tok[s:ntok]
            rel_flat = np.full(need, PAD_ID, dtype=np.float32)
            rel_flat[:avail] = seg[s:ntok].astype(np.float32) - c0
        # out-of-window ids match nothing; keep them far away
        rel_flat = np.where(
            (rel_flat >= 0) & (rel_flat < 256), rel_flat, PAD_ID
        ).astype(np.float32)
        rel = np.ascontiguousarray(rel_flat.reshape(NT2, 128).T)
        inA = (rel >= 0) & (rel < 128)
        inB = (rel >= 128) & (rel < cnts[c])
        tiles_A = np.flatnonzero(inA.any(axis=0))
        tiles_B = np.flatnonzero(inB.any(axis=0))
        if tiles_A.size:
            NA_max = max(NA_max, int(tiles_A[-1]) + 1)
        if tiles_B.size:
            NB0_min = min(NB0_min, int(tiles_B[0]))
        invc = np.ones((128, 2), dtype=np.float32)
        invc[:, 0] = 1.0 / counts_pad[c0 : c0 + 128]
        invc[:, 1] = 1.0 / counts_pad[c0 + 128 : c0 + 256]
        in_maps.append(
            {
                "tok": tok_c if tok_c.flags.c_contiguous else np.ascontiguousarray(tok_c),
                "rel": rel,
                "fcw": fcw,
                "fcb": fcb,
                "invc": invc,
            }
        )
    return in_maps, NT2, NA_max, NB0_min, cnts


def kernel(
    sequence_output,
    fc_w,
    fc_b,
    clause_counts,
    seg_ids,
    n_clauses=NC,
    _loop_iters=0,
    _chunk=4,
    _dma_mode="swdge_unsplit",
    _evac_mode="par",
):
    tok = np.ascontiguousarray(np.asarray(sequence_output, dtype=np.float32)).reshape(
        B * S, H
    )
    fc_w = np.asarray(fc_w, dtype=np.float32)
    fc_b = np.asarray(fc_b, dtype=np.float32)
    counts = np.asarray(clause_counts, dtype=np.float32)
    seg = np.asarray(seg_ids, dtype=np.int32).reshape(-1)

    in_maps, NT2, NA, NB0, cnts = _prepare(tok, seg, counts, fc_w, fc_b)

    key = (NT2, NA, NB0, _loop_iters, _chunk, _dma_mode, _evac_mode)
    nc = _PROGRAM_CACHE.get(key)
    if nc is None:
        nc = _build(
            NT2, NA, NB0, loop_iters=_loop_iters, chunk=_chunk,
            dma_mode=_dma_mode, evac_mode=_evac_mode,
        )
        _PROGRAM_CACHE[key] = nc

    import time

    t0 = time.perf_counter()
    res = bass_utils.run_bass_kernel_spmd(
        nc, in_maps, core_ids=list(range(CORES)), trace=False
    )
    t1 = time.perf_counter()
    LAST_EXEC_INFO.clear()
    LAST_EXEC_INFO.update(
        {
            "wall_s": t1 - t0,
            "NT2": NT2,
            "NA": NA,
            "NB0": NB0,
            "cnts": cnts,
            "nc": nc,
            "in_maps": in_maps,
        }
    )

    shards = [res.results[c]["out"][: cnts[c]] for c in range(CORES)]
    full = np.concatenate(shards, axis=0)[:NC]
    return full.astype(np.float32)
